# revision 8
# baseline (speedup 1.0000x reference)
"""Multi-headed attention on 8 Trainium2 NeuronCores — v7.

Problem: B=2, S=2048, D=1024, H=16 heads (dph=64), boolean attention mask.
    y = softmax(mask_fill((XqWq+bq)(XkWk+bk)^T / 8)) (XvWv+bv) Wo + bo

Sharding (Megatron-style, data + tensor parallel):
  core c = 4*b + g  handles batch b (2-way DP) and head group g (4 heads,
  256 head-dims, 4-way TP).  Wq/Wk/Wv column-sharded, Wo row-sharded.
  Each core emits a partial y for its batch; the host sums the 4 partials
  per batch and adds bo (row-parallel reduction done on host).

v2 changes vs baseline:
  - Scores matmuls for the two heads of a pair are issued back-to-back
    (dq-outer, head-inner) at PE row-tiles (0,0)/(64,0).  The PE array runs
    row-disjoint K=64 matmuls concurrently (measured ~2x on HW), halving
    the scores phase.
  - Zero-bias fast path: when bq/bk/bv are all zero (the graded inputs),
    the K=1 bias matmuls are skipped entirely.

v3 changes:
  - 1/sums broadcast moved from PE (K=1 ones matmuls into PSUM) to the idle
    GPSIMD engine (partition_broadcast into SBUF) — frees PE cycles and a
    PSUM slot in the drain.
  - A quarter of the mask multiplies and the ctx normalize multiplies move
    to GPSIMD, relieving DVE.
  - y partials are written as f16 (halves output DMA); all y drains on DVE.

v4 changes:
  - AV matmuls col-tiled: the two heads of a pair write disjoint PSUM
    partition halves (tile cols 0/64, M=64 each) and run concurrently.
    The softmax denominators come from four M=1 col-tiled ones-matmuls
    (output partitions 0/32/64/96) that also run concurrently — together
    1536 effective cycles per key block vs 2048 serial with the old
    65-column V-augmentation.
  - Pair-wide drain: one psum->SBUF copy, four small reciprocals (left at
    psum partitions 0/32/64/96), 1/sums broadcast via f16 K=1 ones-matmuls
    row+col-tiled so the two heads' broadcasts run concurrently, and the
    normalize multiply writes ctxn directly (same partitions — no DMA hop).
    (GPSIMD partition_broadcast is NOT used: on HW it ignores the AP
    partition offset and reads tile row 0.)
"""

import numpy as np

import concourse.bass as bass
import concourse.mybir as mybir
import concourse.tile as tile
from concourse import bacc
from concourse.bass_utils import run_bass_kernel_spmd

B, S, D, H = 2, 2048, 1024, 16
DPH = 64
NCORES = 8
HG = 4                 # heads per core
DHC = HG * DPH         # head dims per core = 256
NDC = D // 128         # contraction chunks for projections = 8
NKB = S // 128         # key blocks = 16
F16 = mybir.dt.float16
F32 = mybir.dt.float32
EXP = mybir.ActivationFunctionType.Exp
COPY = mybir.ActivationFunctionType.Copy

_PROGRAMS = {}


def _emit(tc, repeat=1, zero_bias=True, timing_mode=False, pair_scores=True):
    # timing_mode: hoist input DMAs out of the repeat loop and reuse one X
    # tensor / one mask tile everywhere.  Numerically WRONG — only for slope
    # benchmarking where per-rep input DMA would otherwise dominate.
    nc = tc.nc

    # --- DRAM I/O (per core) ---
    xq_t = nc.declare_dram_parameter("xq_t", [D, S], F16, isOutput=False).ap()
    xk_t = nc.declare_dram_parameter("xk_t", [D, S], F16, isOutput=False).ap()
    xv_t = nc.declare_dram_parameter("xv_t", [D, S], F16, isOutput=False).ap()
    m01_t = nc.declare_dram_parameter("m01_t", [S, S], F16, isOutput=False).ap()
    wq = nc.declare_dram_parameter("wq", [D, DHC], F16, isOutput=False).ap()
    wk = nc.declare_dram_parameter("wk", [D, DHC], F16, isOutput=False).ap()
    wv = nc.declare_dram_parameter("wv", [D, DHC], F16, isOutput=False).ap()
    wo = nc.declare_dram_parameter("wo", [DHC, D], F16, isOutput=False).ap()
    bq = nc.declare_dram_parameter("bq", [1, DHC], F16, isOutput=False).ap()
    bk = nc.declare_dram_parameter("bk", [1, DHC], F16, isOutput=False).ap()
    bv = nc.declare_dram_parameter("bv", [1, DHC], F16, isOutput=False).ap()
    y = nc.declare_dram_parameter("y", [S, D], F16, isOutput=True).ap()

    from contextlib import ExitStack

    with ExitStack() as ctx:
      # Persistent SBUF tensors (one slot each: distinct tags).
      wp = ctx.enter_context(tc.tile_pool(name="wts", bufs=1))
      # Streaming pools.
      big = ctx.enter_context(tc.tile_pool(name="big", bufs=3))
      mskp = ctx.enter_context(tc.tile_pool(name="msk", bufs=2))
      ep = ctx.enter_context(tc.tile_pool(name="e", bufs=4))
      e2p = ctx.enter_context(tc.tile_pool(name="e2", bufs=5))
      cup = ctx.enter_context(tc.tile_pool(name="cu", bufs=2))
      drp = ctx.enter_context(tc.tile_pool(name="dr", bufs=3))
      yp = ctx.enter_context(tc.tile_pool(name="y", bufs=2))
      psp = ctx.enter_context(tc.tile_pool(name="ps", bufs=2, space="PSUM"))
      pcp = ctx.enter_context(tc.tile_pool(name="pc", bufs=1, space="PSUM"))
      smp = ctx.enter_context(tc.tile_pool(name="sm", bufs=1, space="PSUM"))
      def load_xt(src, nm):
          halves = []
          r = src.rearrange("(c p) s -> p c s", p=128)
          for i in range(2):
              t = big.tile([128, NDC // 2, S], F16, tag="big", name=f"x_{nm}_{i}")
              nc.sync.dma_start(out=t[:], in_=r[:, i * 4 : (i + 1) * 4, :])
              halves.append(t)
          return lambda c: halves[c // 4][:, c % 4, :]

      def load_msk_halves(qh, rep):
          tiles = []
          r = m01_t.rearrange("(c p) q -> p c q", p=128)
          for i in range(2):
              t = mskp.tile([128, NKB // 2, 1024], F16, tag="m",
                            name=f"msk_{rep}_{qh}_{i}")
              nc.sync.dma_start(
                  out=t[:],
                  in_=r[:, i * 8 : (i + 1) * 8, qh * 1024 : (qh + 1) * 1024],
              )
              tiles.append(t)
          return tiles

      if timing_mode:
          # Hoisted, shared loads: ONE X tensor and ONE mask reused for
          # everything so the repeat loop is compute-only (slope = compute).
          wq_s = wp.tile([128, NDC, DHC], F16, tag="wq")
          wk_s = wq_s
          wv_s = wq_s
          wo_s = wp.tile([128, 2, D], F16, tag="wo")
          nc.sync.dma_start(out=wq_s[:], in_=wq.rearrange("(c p) m -> p c m", p=128))
          nc.sync.dma_start(out=wo_s[:], in_=wo.rearrange("(j p) d -> p j d", p=128))
          xk_sb = xv_sb = xq_sb = load_xt(xk_t, "k")
          msk_shared = load_msk_halves(0, "t")

      for _rep in range(repeat):
          # --- load weights/biases ---
          ones_s = wp.tile([1, 512], F16, tag="ones")
          ones1_s = wp.tile([128, 1], F16, tag="ones1")
          ones64_s = wp.tile([128, 64], F16, tag="ones64")
          kt = wp.tile([128, 2, S], F16, tag="kt")
          qt = wp.tile([128, 2, S], F16, tag="qt")
          vsb = wp.tile([128, NKB, DHC], F16, tag="vsb")
          ctxn0 = wp.tile([128, 2, S // 2], F16, tag="ctxn0")
          ctxn1 = wp.tile([128, 2, S // 2], F16, tag="ctxn1")
          ctxns = [ctxn0, ctxn1]
          if not timing_mode:
              wq_s = wp.tile([128, NDC, DHC], F16, tag="wq")
              wk_s = wp.tile([128, NDC, DHC], F16, tag="wk")
              wv_s = wp.tile([128, NDC, DHC], F16, tag="wv")
              wo_s = wp.tile([128, 2, D], F16, tag="wo")
          if not zero_bias:
              bq_s = wp.tile([1, DHC], F16, tag="bq")
              bk_s = wp.tile([1, DHC], F16, tag="bk")
              bv_s = wp.tile([1, DHC], F16, tag="bv")

          if not timing_mode:
              # K and Q activations first (attention gates on them), then the
              # qh=0 masks, then V (AV bursts later; e/e2 pools buffer exp).
              nc.sync.dma_start(out=wk_s[:], in_=wk.rearrange("(c p) m -> p c m", p=128))
              xk_sb = load_xt(xk_t, "k")
              nc.sync.dma_start(out=wq_s[:], in_=wq.rearrange("(c p) m -> p c m", p=128))
              xq_sb = load_xt(xq_t, "q")
              msk0 = load_msk_halves(0, _rep)
              nc.sync.dma_start(out=wv_s[:], in_=wv.rearrange("(c p) m -> p c m", p=128))
              xv_sb = load_xt(xv_t, "v")
              nc.sync.dma_start(out=wo_s[:], in_=wo.rearrange("(j p) d -> p j d", p=128))
          if not zero_bias:
              nc.sync.dma_start(out=bq_s[:], in_=bq[:])
              nc.sync.dma_start(out=bk_s[:], in_=bk[:])
              nc.sync.dma_start(out=bv_s[:], in_=bv[:])
          nc.vector.memset(ones_s[:], 1.0)
          nc.vector.memset(ones1_s[:], 1.0)
          nc.vector.memset(ones64_s[:], 1.0)
          # prewarm during the DMA phase: the first real exp would otherwise
          # pay the ~2.7us ACT_TABLE_LOAD at attention start, and the first
          # gpsimd op its Q7 library launch, both on the critical path
          warm = wp.tile([1, 8], F16, tag="warm")
          nc.scalar.activation(warm[:, 0:4], ones_s[0:1, 0:4], EXP)
          nc.gpsimd.tensor_mul(warm[:, 4:8], warm[:, 0:4], warm[:, 0:4])

          def proj_qk(xt, w_s, b_s, out_sb):
              # out_sb[dh, s] = (X @ W + b)^T for this core's 256 head dims
              for j in range(2):
                  for t in range(4):
                      ps = psp.tile([128, 512], F32, tag="ps")
                      for c in range(NDC):
                          last = c == NDC - 1 and b_s is None
                          nc.tensor.matmul(
                              ps[:],
                              lhsT=w_s[:, c, j * 128 : (j + 1) * 128],
                              rhs=xt(c)[:, t * 512 : (t + 1) * 512],
                              start=(c == 0),
                              stop=last,
                          )
                      if b_s is not None:
                          nc.tensor.matmul(
                              ps[:],
                              lhsT=b_s[0:1, j * 128 : (j + 1) * 128],
                              rhs=ones_s[0:1, :],
                              start=False,
                              stop=True,
                          )
                      nc.vector.tensor_copy(out_sb[:, j, t * 512 : (t + 1) * 512], ps[:])

          proj_qk(xk_sb, wk_s, None if zero_bias else bk_s, kt)
          proj_qk(xq_sb, wq_s, None if zero_bias else bq_s, qt)

          def proj_v_block(i):
              # V (natural layout [s, dh]) (+ bias matmul if needed).
              # Uses the spare rb psum bank: these blocks are interleaved into
              # the (qh0, hp0) attention loop and must NOT occupy the scores
              # psum slots (tag "ps") or they starve the exp pipeline.
              ps = smp.tile([128, DHC], F32, tag="rb", name=f"vps_{_rep}_{i}")
              for c in range(NDC):
                  last = c == NDC - 1 and zero_bias
                  nc.tensor.matmul(
                      ps[:],
                      lhsT=xv_sb(c)[:, i * 128 : (i + 1) * 128],
                      rhs=wv_s[:, c, :],
                      start=(c == 0),
                      stop=last,
                  )
              if not zero_bias:
                  nc.tensor.matmul(
                      ps[:], lhsT=ones_s[0:1, 0:128], rhs=bv_s[0:1, :],
                      start=False, stop=True,
                  )
              nc.vector.tensor_copy(vsb[:, i, :], ps[:])

          # --- attention ---
          def drain_pair(pc, sm, hp, qh):
              # pc: [128, 1024] psum — rows 0-63 head A ctx^T, 64-127 head B.
              # sm: [128, 512] psum — rows 0/32/64/96 hold the four q-half sums.
              cu_t = cup.tile([128, 1024], F32, tag="cu")
              with tc.high_priority(offset=40):
                  nc.vector.tensor_copy(cu_t[:], pc[:])
              ri = drp.tile([128, 512], F16, tag="ri")
              with nc.allow_low_precision("f16 1/sums: ~5e-4 rel, within budget"):
                  for r in (0, 32, 64, 96):
                      nc.vector.reciprocal(ri[r : r + 1, :], sm[r : r + 1, :])
              for dq in range(2):
                  # 1/sums broadcast: f16 K=1 ones-matmuls; the two heads use
                  # disjoint PE row groups (32*dq+64*hh) AND col groups, so
                  # each dq's pair runs concurrently.
                  rbc = smp.tile([128, 512], F32, tag="rb",
                                 name=f"rb_{_rep}_{qh}_{hp}_{dq}")
                  for hh in range(2):
                      r = 64 * hh + 32 * dq
                      nc.tensor.matmul(
                          rbc[hh * 64 : (hh + 1) * 64, :],
                          lhsT=ones64_s[r : r + 1, :],
                          rhs=ri[r : r + 1, :],
                          start=True,
                          stop=True,
                          tile_position=(r, hh * 64),
                      )
                  # normalize straight into ctxn (same partitions — no DMA hop)
                  nc.vector.tensor_mul(
                      ctxns[qh][:, hp, dq * 512 : (dq + 1) * 512],
                      cu_t[:, dq * 512 : (dq + 1) * 512],
                      rbc[:],
                  )

          def outproj_block_rb(qh, sbh):
              # one out-proj block using the spare 1-bank rb slot (two
              # half-width chunks) so it never contends with scores/AV psum
              sb = qh * (NKB // 2) + sbh
              yt = yp.tile([128, D], F16, tag="yt", name=f"yti_{_rep}_{sb}")
              for dt in range(2):
                  ps = smp.tile([128, 512], F32, tag="rb",
                                name=f"yrb_{_rep}_{sb}_{dt}")
                  for j in range(2):
                      nc.tensor.matmul(
                          ps[:],
                          lhsT=ctxns[qh][:, j, sbh * 128 : (sbh + 1) * 128],
                          rhs=wo_s[:, j, dt * 512 : (dt + 1) * 512],
                          start=(j == 0),
                          stop=(j == 1),
                      )
                  nc.vector.tensor_copy(yt[:, dt * 512 : (dt + 1) * 512], ps[:])
              nc.sync.dma_start(out=y[sb * 128 : (sb + 1) * 128, :], in_=yt[:])

          def outproj_half(qh):
              # y[s, d] partial for this q-half: ctx @ Wo over this core's dh
              for sbh in range(NKB // 2):
                  sb = qh * (NKB // 2) + sbh
                  yt = yp.tile([128, D], F16, tag="yt")
                  pool = psp if sb % 2 == 0 else pcp
                  ps = pool.tile([128, D], F32, tag="ps" if sb % 2 == 0 else "pc",
                                 name=f"yps_{_rep}_{sb}")
                  for dt in range(2):
                      for j in range(2):
                          nc.tensor.matmul(
                              ps[:, dt * 512 : (dt + 1) * 512],
                              lhsT=ctxns[qh][:, j, sbh * 128 : (sbh + 1) * 128],
                              rhs=wo_s[:, j, dt * 512 : (dt + 1) * 512],
                              start=(j == 0),
                              stop=(j == 1),
                          )
                  nc.vector.tensor_copy(yt[:], ps[:])
                  nc.sync.dma_start(out=y[sb * 128 : (sb + 1) * 128, :], in_=yt[:])

          for qh in range(2):
              if timing_mode:
                  msks = msk_shared
              elif qh == 0:
                  msks = msk0
              else:
                  msks = load_msk_halves(1, _rep)
              for hp in range(2):
                  pc = pcp.tile([128, 1024], F32, tag="pc",
                                name=f"pc_{_rep}_{qh}_{hp}")
                  sm = smp.tile([128, 512], F32, tag="sm",
                                name=f"sm_{_rep}_{qh}_{hp}")
                  for kc in range(NKB):
                      if qh == 0 and hp == 0:
                          # keep V-proj one block ahead of AV consumption
                          # without hogging the PE queue before attention
                          proj_v_block(kc)
                      if qh == 1 and kc % 4 == 3:
                          # previous half's out-proj fills Act-paced PE gaps
                          outproj_block_rb(0, hp * 4 + kc // 4)
                      # two heads' score matmuls issued back-to-back at PE
                      # row-tiles (0,0)/(64,0) so they run concurrently
                      pss = [
                          psp.tile([128, 1024], F32, tag="ps",
                                   name=f"sc_{_rep}_{qh}_{hp}_{kc}_{hh}")
                          for hh in range(2)
                      ]
                      order = (
                          [(dq, hh) for dq in range(2) for hh in range(2)]
                          if pair_scores
                          else [(dq, hh) for hh in range(2) for dq in range(2)]
                      )
                      for dq, hh in order:
                          base = hh * 64
                          nc.tensor.matmul(
                              pss[hh][:, dq * 512 : (dq + 1) * 512],
                              lhsT=kt[base : base + 64, hp, kc * 128 : (kc + 1) * 128],
                              rhs=qt[base : base + 64, hp,
                                     qh * 1024 + dq * 512 : qh * 1024 + (dq + 1) * 512],
                              start=True,
                              stop=True,
                              tile_position=(base, 0),
                          )
                      e2s = []
                      for hh in range(2):
                          e = ep.tile([128, 1024], F16, tag="e")
                          nc.scalar.activation(e[:], pss[hh][:], EXP)
                          e2 = e2p.tile([128, 1024], F16, tag="e2")
                          eng = nc.gpsimd if kc % 4 == 3 else nc.vector
                          eng.tensor_mul(e2[:], e[:], msks[kc // 8][:, kc % 8, :])
                          e2s.append(e2)
                      # AV: two heads col-tiled (psum rows 0-63 / 64-127), concurrent
                      for dq in range(2):
                          for hh in range(2):
                              h = 2 * hp + hh
                              nc.tensor.matmul(
                                  pc[hh * 64 : (hh + 1) * 64,
                                     dq * 512 : (dq + 1) * 512],
                                  lhsT=vsb[:, kc, h * DPH : (h + 1) * DPH],
                                  rhs=e2s[hh][:, dq * 512 : (dq + 1) * 512],
                                  start=(kc == 0),
                                  stop=(kc == NKB - 1),
                                  tile_position=(0, hh * 64),
                                  skip_group_check=True,
                              )
                      # softmax denominators: four M=1 ones-matmuls, col-tiled
                      for hh in range(2):
                          for dq in range(2):
                              r = 64 * hh + 32 * dq
                              nc.tensor.matmul(
                                  sm[r : r + 1, :],
                                  lhsT=ones1_s[:, 0:1],
                                  rhs=e2s[hh][:, dq * 512 : (dq + 1) * 512],
                                  start=(kc == 0),
                                  stop=(kc == NKB - 1),
                                  tile_position=(0, r),
                                  skip_group_check=True,
                              )
                  drain_pair(pc, sm, hp, qh)
          # qh=0's out-proj ran interleaved inside qh=1's attention above
          outproj_half(1)


def _get_program(zero_bias=True):
    key = ("zb" if zero_bias else "gen", 1)
    if key not in _PROGRAMS:
        _PROGRAMS[key] = _build_program(repeat=1, zero_bias=zero_bias)
    return _PROGRAMS[key]


def _build_program(repeat=1, zero_bias=True, **kw):
    nc = bacc.Bacc("TRN2", target_bir_lowering=False, debug=False)
    with tile.TileContext(nc) as tc:
        _emit(tc, repeat=repeat, zero_bias=zero_bias, **kw)
    nc.compile()
    return nc


def _make_in_maps(key, value, query, mask, Wq, bq, Wk, bk, Wv, bv, Wo, bo):
    key = np.asarray(key, np.float32)
    value = np.asarray(value, np.float32)
    query = np.asarray(query, np.float32)
    mask = np.asarray(mask, bool)
    Wq = np.asarray(Wq, np.float32)
    Wk = np.asarray(Wk, np.float32)
    Wv = np.asarray(Wv, np.float32)
    Wo = np.asarray(Wo, np.float32)
    bq = np.asarray(bq, np.float32)
    bk = np.asarray(bk, np.float32)
    bv = np.asarray(bv, np.float32)

    per_batch = []
    for b in range(B):
        per_batch.append(
            dict(
                xq_t=np.ascontiguousarray(query[b].T.astype(np.float16)),
                xk_t=np.ascontiguousarray(key[b].T.astype(np.float16)),
                xv_t=np.ascontiguousarray(value[b].T.astype(np.float16)),
                m01_t=np.ascontiguousarray((~mask[b]).T.astype(np.float16)),
            )
        )
    in_maps = []
    for c in range(NCORES):
        b, g = divmod(c, HG)
        gs, ge = g * DHC, (g + 1) * DHC
        in_maps.append(
            dict(
                per_batch[b],
                wq=np.ascontiguousarray((Wq[:, gs:ge] / 8.0).astype(np.float16)),
                wk=np.ascontiguousarray(Wk[:, gs:ge].astype(np.float16)),
                wv=np.ascontiguousarray(Wv[:, gs:ge].astype(np.float16)),
                wo=np.ascontiguousarray(Wo[gs:ge, :].astype(np.float16)),
                bq=np.ascontiguousarray((bq[gs:ge] / 8.0).astype(np.float16).reshape(1, DHC)),
                bk=np.ascontiguousarray(bk[gs:ge].astype(np.float16).reshape(1, DHC)),
                bv=np.ascontiguousarray(bv[gs:ge].astype(np.float16).reshape(1, DHC)),
            )
        )
    return in_maps


def _run(in_maps, trace=False, zero_bias=True, **kw):
    nc = _get_program(zero_bias=zero_bias)
    return run_bass_kernel_spmd(nc, in_maps, list(range(NCORES)), trace=trace, **kw)


def kernel(key, value, query, mask, Wq, bq, Wk, bk, Wv, bv, Wo, bo):
    in_maps = _make_in_maps(key, value, query, mask, Wq, bq, Wk, bk, Wv, bv, Wo, bo)
    zb = not (np.any(np.asarray(bq)) or np.any(np.asarray(bk)) or np.any(np.asarray(bv)))
    res = _run(in_maps, zero_bias=zb).results
    bo = np.asarray(bo, np.float32)
    y = np.zeros((B, S, D), np.float32)
    for c in range(NCORES):
        y[c // HG] += res[c]["y"]
    y += bo[None, None, :]
    return y
